# revision 1
# baseline (speedup 1.0000x reference)
"""DenseKAN forward as a single fused matmul on TRN2.

Math: the reference uses a uniform knot grid (spacing h=0.4 on
[-2.2, 2.2]), so the Cox-de Boor bases are shifted copies of the
cardinal cubic B-spline with u = 2.5x + 5.5 in [3, 8):

    B_j(x) = Q(u - j),   Q(s) = (1/6) sum_m (-1)^m C(4,m) relu(s-m)^3

Using Q's symmetry Q(s) = Q(4-s), each basis is expanded from the side
that keeps the truncated-power features small (bounded by ~26 after the
1/2.5 rescale, which keeps the binomial cancellation mild enough for
the PE's reduced-precision fp32r mode):

    blocks 0..3:  f_n = max((n-1.5)/2.5 - x, 0)^3   (right-side powers)
    blocks 4..7:  f_n = max(x + (5.5-n)/2.5, 0)^3   (left-side powers)
    block  8:     silu(x)

    B_0 = 2.5^3/6 * f_0            B_7 = 2.5^3/6 * f_7
    B_1 = 2.5^3/6 * (f_1 - 4 f_0)  B_6 = 2.5^3/6 * (f_6 - 4 f_7)  etc.

All coefficients, the per-dim scale factor, and the bias (via partition
of unity, sum_j B_j == 1) are folded into the weights on the host, so
the whole layer is out = F(x) @ W2 with F computed on-chip:
per block one GpSimd dual-op (add,max), one ACT Square, one DVE mul.
The host also pre-transposes x (shipping [x^T | -x^T]) so no on-chip
transpose is needed. Batch is sharded across the 8 cores (128 rows
each); weights are replicated.
"""

import numpy as np

import concourse.bass as bass
import concourse.mybir as mybir
import concourse.tile as tile
from concourse import bacc
from concourse.bass_utils import run_bass_kernel_spmd

BATCH = 1024
IN = 256
UNITS = 256
GK = 8  # number of spline bases per input dim
NF = GK + 1  # + silu feature block
K = IN * NF  # 2304 contraction rows
N_CORES = 8
BS = BATCH // N_CORES  # 128 batch rows per core
KT = K // 128  # 18 K-tiles
W_CHUNKS = (2, 4, 6, 6)
N_WARM = 6  # PE warm-up matmuls (HAM clock-gate burn-in)

FP32 = mybir.dt.float32
MM_DT = mybir.dt.float32r  # matmul compute dtype (fp32 bit layout)

AluOp = mybir.AluOpType

_cache = {}


def _build():
    nc = bacc.Bacc("TRN2", target_bir_lowering=False, debug=False,
                   enable_asserts=False, num_devices=N_CORES)
    # host ships [x^T | -x^T] as the SBUF image: (128, 4*BS)
    xt_d = nc.dram_tensor("xt", [128, 4 * BS], FP32,
                          kind="ExternalInput").ap()
    # host pre-swizzled: w2[p, k, o] = W2_flat[128*k + p, o]
    w_d = nc.dram_tensor("w2", [128, KT, UNITS], MM_DT,
                         kind="ExternalInput").ap()
    o_d = nc.dram_tensor("out", [BS, UNITS], FP32, kind="ExternalOutput").ap()

    with tile.TileContext(nc) as tc:
        with (
            tc.tile_pool(name="const", bufs=1) as cpool,
            tc.tile_pool(name="blk", bufs=3) as bpool,
            tc.tile_pool(name="psum", bufs=1, space="PSUM") as ppool,
        ):
            # x first: the whole feature pipeline hangs off it
            xt = cpool.tile([128, 4 * BS], FP32)
            nc.sync.dma_start(xt[:], xt_d[:])

            # weights stream behind x; first chunk small so the PE can
            # start on the silu block early
            w2 = cpool.tile([128, KT, UNITS], MM_DT)
            lo = 0
            for sz in W_CHUNKS:
                nc.sync.dma_start(w2[:, lo:lo + sz, :], w_d[:, lo:lo + sz, :])
                lo += sz

            # PE warm-up: HAM keeps the PE at 1.2 GHz until ~3.4us of
            # sustained activity; burn that in while the weights stream
            wtile = cpool.tile([128, 512], MM_DT)
            nc.vector.tensor_copy(
                wtile[:], nc.const_aps.tensor(1.0, (128, 512), FP32))
            wpsum = ppool.tile([128, 512], FP32)
            for _ in range(N_WARM):
                nc.tensor.matmul(wpsum[:], wtile[:, 0:128], wtile[:],
                                 start=True, stop=True)

            T = cpool.tile([128, NF * 256], MM_DT)
            opsum = ppool.tile([BS, UNITS], FP32)

            # weight k-tile order (host side matches): silu pair first,
            # then feature blocks in compute order
            nc.scalar.activation(T[:, GK * 256:(GK + 1) * 256],
                                 xt[:, 0:2 * BS],
                                 mybir.ActivationFunctionType.Silu)
            nc.tensor.matmul(opsum[:], T[:, 2048:2176], w2[:, 0, :],
                             start=True, stop=False)
            nc.tensor.matmul(opsum[:], T[:, 2176:2304], w2[:, 1, :],
                             start=False, stop=False)

            for n in range(GK):
                if n < 4:
                    src = xt[:, 2 * BS:4 * BS]  # -x^T
                    c = (n - 1.5) / 2.5
                else:
                    src = xt[:, 0:2 * BS]  # x^T
                    c = (5.5 - n) / 2.5
                t1 = bpool.tile([128, 256], FP32, tag="t1")
                nc.gpsimd.tensor_scalar(t1[:], src, float(c), 0.0,
                                        AluOp.add, AluOp.max)
                sq = bpool.tile([128, 256], FP32, tag="sq")
                nc.scalar.square(sq[:], t1[:])
                blk = T[:, n * 256:(n + 1) * 256]
                nc.vector.tensor_mul(blk, sq[:], t1[:])
                for h in range(2):
                    k = 2 * n + h
                    nc.tensor.matmul(opsum[:],
                                     T[:, k * 128:(k + 1) * 128],
                                     w2[:, 2 + k, :],
                                     start=False, stop=(k == 2 * GK - 1))

            osb = cpool.tile([BS, UNITS], FP32)
            nc.vector.tensor_copy(osb[:], opsum[:])
            nc.sync.dma_start(o_d[:], osb[:])

    nc.compile()
    return nc


def _fold_weights(spline_kernel, scale_factor, bias):
    """-> (128, KT, UNITS) swizzled folded weights, w2[p,k,o]=W2[128k+p,o]."""
    sk = spline_kernel.astype(np.float64)
    sf = scale_factor.astype(np.float64)
    b = bias.astype(np.float64)
    # W[i,j,o] = sk*sf + bias/IN  (bias folded via sum_j B_j == 1)
    W = sk * sf[:, None, :] + b[None, None, :] / IN
    comb = 2.5 ** 3 * np.array([1.0, -4.0, 6.0, -4.0, 1.0]) / 6.0
    # A[j, n] = coefficient of feature-block n in basis j
    A = np.zeros((GK, GK))
    for j in range(4):  # right-side: B_j = sum_m comb[m] * f_{j-m}
        for m in range(j + 1):
            A[j, j - m] = comb[m]
    for j in range(4, GK):  # left-side: B_j = sum_m comb[m] * f_{j+m}
        for m in range(GK - j):
            A[j, j + m] = comb[m]
    W2 = np.einsum("jn,ijo->nio", A, W)  # (GK, IN, UNITS)
    Wfull = np.concatenate([sf[None, :, :], W2], axis=0)  # silu block first
    flat = Wfull.reshape(K, UNITS)
    sw = flat.reshape(KT, 128, UNITS).transpose(1, 0, 2)  # -> [p, k, o]
    return np.ascontiguousarray(sw.astype(np.float32))


def _prep_x(x):
    """(BATCH, IN) -> per-core (128, 4*BS) SBUF images [x^T | -x^T]."""
    x = np.asarray(x, dtype=np.float32)
    outs = []
    for c in range(N_CORES):
        xs = x[c * BS:(c + 1) * BS]  # (BS, IN)
        xtc = np.ascontiguousarray(xs.T)  # (IN, BS)
        b0, b1 = xtc[:128], xtc[128:]
        outs.append(np.ascontiguousarray(
            np.concatenate([b0, b1, -b0, -b1], axis=1)))  # (128, 4*BS)
    return outs


def kernel(x, spline_kernel, scale_factor, bias):
    if "nc" not in _cache:
        _cache["nc"] = _build()
    nc = _cache["nc"]

    w2 = _fold_weights(spline_kernel, scale_factor, bias)
    xts = _prep_x(x)
    in_maps = [{"xt": xts[c], "w2": w2} for c in range(N_CORES)]
    res = run_bass_kernel_spmd(nc, in_maps, list(range(N_CORES)))
    out = np.concatenate([res.results[c]["out"] for c in range(N_CORES)],
                         axis=0)
    return out.astype(np.float32)



# revision 6
# speedup vs baseline: 2.4679x; 2.4679x over previous
"""DenseKAN forward as a single fused matmul on TRN2.

Math: the reference's uniform knot grid gives cardinal cubic B-splines
B_j(x) = Q(u - j) with u = 2.5x + 5.5 in [3, 8).  In the truncated-power
form Q(s) = (1/6) sum_m (-1)^m C(4,m) relu(s-m)^3, every knot k <= 3
satisfies u >= k on the whole domain, so those terms are plain cubics in
x and collapse into the global polynomial {1, x, x^2, x^3}.  Only knots
k = 4..7 keep the relu:

    features per input dim: [x, x^2, x^3, g4, g5, g6, g7, silu(x)]
    g_k(x) = relu(x + c_k)^3,  c_k = (5.5-k)/2.5 in {0.6, 0.2, -0.2, -0.6}

(7 spline features instead of 9; the constant feature plus the layer
bias is injected via one matmul against an all-ones stationary tile.)
Everything else folds into the weights on the host (float64), so the
layer is out = F(x) @ W with F computed on-chip in bf16:

    ACT: q_k = Square(x + c_k) (fused bias), silu
    DVE: cast x, x^2 = x*x, x^3 = x^2*x, r_k = max(x+c_k, 0), g_k = q_k*r_k

fp16 keeps the DVE in its 2x packed modes, halves the weight DMA, and
holds quantization to ~5e-3 (bf16's 8 mantissa bits gave 3.7e-2).
GpSimd does nothing (its tensor_scalar runs ~15x below DVE here).
Batch is sharded across the 8 cores (128 rows each); weights replicated.
"""

import math

import numpy as np

import concourse.bass as bass
import concourse.mybir as mybir
import concourse.tile as tile
from concourse import bacc
from concourse.bass_utils import run_bass_kernel_spmd

BATCH = 1024
IN = 256
UNITS = 256
N_CORES = 8
BS = BATCH // N_CORES  # 128 batch rows per core

NB = 8  # T feature blocks: x, x2, x3, g4..g7, silu
KT = 1 + 2 * NB  # bias k-tile + 16 feature k-tiles
CS = (0.6, 0.2, -0.2, -0.6)  # biases for g4..g7
N_WARM = 7  # PE warm-up matmuls (HAM clock ramp)
W_CHUNKS = ((0, 5), (5, 11), (11, 17))

FP32 = mybir.dt.float32
FP16 = mybir.dt.float16
AluOp = mybir.AluOpType
Act = mybir.ActivationFunctionType

_cache = {}


def _build():
    nc = bacc.Bacc("TRN2", target_bir_lowering=False, debug=False,
                   enable_asserts=False, num_devices=N_CORES)
    # x^T as two dim-halves side by side: col c -> (dim 128*(c//128)+p, batch c%128)
    xt_d = nc.dram_tensor("xt", [128, 2 * BS], FP32, kind="ExternalInput").ap()
    # w[p, 0, o] = bias row; w[p, 1+k, o] = folded weights for feature k-tile k
    w_d = nc.dram_tensor("w2", [128, KT, UNITS], FP16, kind="ExternalInput").ap()
    o_d = nc.dram_tensor("out", [BS, UNITS], FP32, kind="ExternalOutput").ap()

    with tile.TileContext(nc) as tc:
        with (
            tc.tile_pool(name="const", bufs=1) as cpool,
            tc.tile_pool(name="psum", bufs=1, space="PSUM") as ppool,
        ):
            xt = cpool.tile([128, 2 * BS], FP32)
            w2 = cpool.tile([128, KT, UNITS], FP16)
            wt = cpool.tile([128, 512], FP16)
            T = cpool.tile([128, NB * 256], FP16)
            qs = [cpool.tile([128, 256], FP16, name=f"q{m}")
                  for m in range(4)]
            rs = [cpool.tile([128, 256], FP16, name=f"r{m}")
                  for m in range(4)]
            osb = cpool.tile([BS, UNITS], FP32)
            cbias = [cpool.tile([128, 1], FP32, name=f"c{m}")
                     for m in range(4)]
            wpsum = ppool.tile([128, 512], FP32)
            opsum = ppool.tile([BS, UNITS], FP32)

            for m in range(4):
                nc.gpsimd.memset(cbias[m][:], float(CS[m]))

            # x first, then weights in consume order
            nc.sync.dma_start(xt[:], xt_d[:])
            for lo, hi in W_CHUNKS:
                nc.sync.dma_start(w2[:, lo:hi, :], w_d[:, lo:hi, :])

            # all-ones tile: warm-up fodder + stationary for the bias matmul
            nc.vector.memset(wt[:], 1.0)
            for _ in range(N_WARM):
                nc.tensor.matmul(wpsum[:], wt[:, 0:128], wt[:],
                                 start=True, stop=True)
            # opsum[b, o] = sum_p w2[p, 0, o] = bias_o
            nc.tensor.matmul(opsum[:], wt[:, 0:128], w2[:, 0, :],
                             start=True, stop=False)

            def mm(b, stop=False):
                for h in range(2):
                    k = 2 * b + h
                    nc.tensor.matmul(opsum[:], T[:, k * 128:(k + 1) * 128],
                                     w2[:, 1 + k, :],
                                     start=False, stop=stop and h == 1)

            blk = [T[:, b * 256:(b + 1) * 256] for b in range(NB)]

            # ACT queue: q4..q7 (Square with fused +c), then silu
            for m in range(4):
                nc.scalar.activation(qs[m][:], xt[:], Act.Square,
                                     bias=cbias[m][:])
            nc.scalar.activation(blk[7], xt[:], Act.Silu)

            # DVE queue (bf16 2x modes), interleaved with PE consumption
            nc.vector.tensor_copy(blk[0], xt[:])          # x cast
            mm(0)
            nc.vector.tensor_mul(blk[1], blk[0], blk[0])  # x^2
            mm(1)
            nc.vector.tensor_mul(blk[2], blk[1], blk[0])  # x^3
            mm(2)
            for m in range(4):
                nc.vector.tensor_scalar(rs[m][:], xt[:], float(CS[m]), 0.0,
                                        AluOp.add, AluOp.max)
                nc.vector.tensor_mul(blk[3 + m], qs[m][:], rs[m][:])
                mm(3 + m)
            mm(7, stop=True)  # silu block

            nc.scalar.copy(osb[:], opsum[:])
            nc.sync.dma_start(o_d[:], osb[:])

    nc.compile()
    return nc


def _coef_matrices():
    """P[j,d]: coeff of x^d in B_j's polynomial part; R[j,m]: coeff of g_{4+m}."""
    P = np.zeros((NB, 4))
    R = np.zeros((NB, 4))
    for j in range(NB):
        for m in range(5):
            k = j + m
            s = (-1) ** m * math.comb(4, m) / 6.0
            if k <= 3:
                for d in range(4):
                    P[j, d] += s * math.comb(3, d) * 2.5 ** d * (5.5 - k) ** (3 - d)
            elif k <= 7:
                R[j, k - 4] += s * 2.5 ** 3
    return P, R


def _fold_weights(spline_kernel, scale_factor, bias):
    """-> (128, KT, UNITS) bf16: slot 0 = bias row, slots 1.. = feature k-tiles."""
    sk = spline_kernel.astype(np.float64)
    sf = scale_factor.astype(np.float64)
    b = bias.astype(np.float64)
    V = sk * sf[:, None, :]  # (in, 8, out)
    P, R = _coef_matrices()
    Wpoly = np.einsum("jd,ijo->dio", P, V)  # const, x, x^2, x^3
    Wg = np.einsum("jm,ijo->mio", R, V)  # g4..g7
    c = b + Wpoly[0].sum(axis=0)  # (out,)
    feats = np.concatenate([Wpoly[1:], Wg, sf[None]], axis=0)  # (8, in, out)
    kt = feats.reshape(2 * NB, 128, UNITS)  # k-tile k, dim row p
    full = np.concatenate([np.broadcast_to(c / 128.0, (1, 128, UNITS)), kt], 0)
    sw = full.transpose(1, 0, 2)  # -> [p, slot, o]
    return np.ascontiguousarray(sw.astype(np.float32).astype(np.float16))


def _prep_x(x):
    """(BATCH, IN) -> per-core (128, 2*BS) SBUF images [x^T half | half]."""
    x = np.asarray(x, dtype=np.float32)
    outs = []
    for cid in range(N_CORES):
        xtc = np.ascontiguousarray(x[cid * BS:(cid + 1) * BS].T)  # (IN, BS)
        outs.append(np.ascontiguousarray(
            np.concatenate([xtc[:128], xtc[128:]], axis=1)))  # (128, 2*BS)
    return outs


def kernel(x, spline_kernel, scale_factor, bias):
    if "nc" not in _cache:
        _cache["nc"] = _build()
    nc = _cache["nc"]

    w2 = _fold_weights(spline_kernel, scale_factor, bias)
    xts = _prep_x(x)
    in_maps = [{"xt": xts[c], "w2": w2} for c in range(N_CORES)]
    res = run_bass_kernel_spmd(nc, in_maps, list(range(N_CORES)))
    out = np.concatenate([res.results[c]["out"] for c in range(N_CORES)],
                         axis=0)
    return out.astype(np.float32)
